# revision 30
# baseline (speedup 1.0000x reference)
"""Distributed multi-head attention kernel for 8 Trainium2 NeuronCores.

Problem: B=4, N=2048, E=1024, H=16 heads (head_dim 64), QKV + out projections.
Sharding: core c handles batch b=c//2 and head-group g=c%2 (8 heads = D-slice
of 512). QKV projections are column-sharded, the out projection is row-sharded;
the two partial outputs per batch are summed on the host during unshard.

Per-core dataflow (all matmuls bf16 with fp32 PSUM accumulation):
  A) QpT/KpT [512, 2048] and Vp [2048, 512] projections. Host pre-transposes
     q/k/v to [E, N] so the contraction dim lands on SBUF partitions.
  B) S^T[j, i] = Kp_h^T.T @ Qp_h^T per head. K=64, so head pairs are packed
     onto PE row-groups 0-63 / 64-127 (partition-base-derived tile_position).
     exp(scale*x) is fused into the PSUM->SBUF eviction on the scalar engine
     (no max-subtraction: logits are O(1) by construction).
  C) O^T_aug[65, i] accumulates Vp_aug^T @ expS^T over j-tiles, where Vp is
     augmented with a ones column so row 64 of the product is the softmax
     denominator Z.
  D) Normalize: evict O_aug to SBUF (frees the PSUM bank early), DMA the Z row
     to partition 0, reciprocal_approx_fast, gpsimd partition_broadcast, one
     multiply per head; stack head pairs (one SBUF->SBUF DMA partition shift),
     out-project, store out^T partial [1024, 2048].
"""

import sys

import numpy as np


def _ensure_paths():
    try:
        import concourse.bass  # noqa: F401
    except ImportError:
        for p in ("/opt/trn_rl_repo",):
            if p not in sys.path:
                sys.path.insert(0, p)
        import concourse.bass  # noqa: F401


_ensure_paths()

import ml_dtypes  # noqa: E402
import concourse.bass as bass  # noqa: E402
import concourse.bacc as bacc  # noqa: E402
import concourse.mybir as mybir  # noqa: E402
import concourse.tile as tile  # noqa: E402
from concourse.bass_utils import run_bass_kernel_spmd  # noqa: E402

BF16NP = ml_dtypes.bfloat16

B, N, E = 4, 2048, 1024
H, HD = 16, 64
G = 2                 # head-group (tensor-parallel) factor
S = E // G            # 512: per-core slice of the internal dim
HPC = H // G          # 8 heads per core
ET = E // 128         # 8 contraction tiles for the projections
DT = S // 128         # 4 d-tiles per core (= head pairs)
NT = N // 128         # 16 n-tiles
NBLK = N // 512       # 4 n/i blocks
SCALE = 1.0 / float(np.sqrt(HD))
# Schraudolph exp in bf16-bit space: bf16_bits(exp(s*SCALE)) ~= s*EXP_A + EXP_B
# (DVE f32->int16 convert rounds to nearest; B tuned for ~zero mean bias)
EXP_A = 128.0 * 1.4426950408889634 * SCALE
EXP_B = 127.0 * 128.0 - 7.2
# j-tiles (of 16 per block) whose exp runs on DVE via Schraudolph; the rest
# use the scalar engine's exact exp. Balances ACT vs DVE load.
DVE_JT = frozenset((1, 4, 6, 9, 11, 14))

last_exec_time_ns = None
last_results = None


def _install_ntff_shim():
    """Register the axon NTFF profile hook bass_utils wants under trace=True."""
    import types

    if "antenv.axon_hooks" in sys.modules:
        return
    mod = types.ModuleType("antenv.axon_hooks")
    _h = [None]
    mod.set_axon_ntff_profile_hook = lambda h: _h.__setitem__(0, h)
    mod.get_axon_ntff_profile_hook = lambda: _h[0]
    try:
        import antenv

        sys.modules["antenv.axon_hooks"] = mod
        antenv.axon_hooks = mod
        from trn_agent_boot.trn_boot import _ntff_profile_via_ctypes

        hook = _ntff_profile_via_ctypes("/opt/axon/libaxon_pjrt.so")
        if hook is not None:
            mod.set_axon_ntff_profile_hook(hook)
    except Exception:
        pass


def _build(has_bq, has_bk, has_bv, has_bo):
    f32 = mybir.dt.float32
    bf16 = mybir.dt.bfloat16
    PSUM = bass.MemorySpace.PSUM

    nc = bacc.Bacc("TRN2", target_bir_lowering=False, debug=False)

    # host pre-tiles all inputs partition-major: [p, et, ...] so each DMA
    # moves one contiguous multi-KB line per partition
    qT_ext = nc.declare_dram_parameter("qT", [128, ET * N], bf16, isOutput=False)
    kT_ext = nc.declare_dram_parameter("kT", [128, ET * N], bf16, isOutput=False)
    vT_ext = nc.declare_dram_parameter("vT", [128, ET * N], bf16, isOutput=False)
    wq_ext = nc.declare_dram_parameter("wq", [128, ET * S], bf16, isOutput=False)
    wk_ext = nc.declare_dram_parameter("wk", [128, ET * S], bf16, isOutput=False)
    wv_ext = nc.declare_dram_parameter("wv", [128, ET * S], bf16, isOutput=False)
    wo_ext = nc.declare_dram_parameter("wo", [128, DT * E], bf16, isOutput=False)
    bq_ext = nc.declare_dram_parameter("bq", [1, S], bf16, isOutput=False) if has_bq else None
    bk_ext = nc.declare_dram_parameter("bk", [1, S], bf16, isOutput=False) if has_bk else None
    bv_ext = nc.declare_dram_parameter("bv", [1, S], bf16, isOutput=False) if has_bv else None
    bo_ext = nc.declare_dram_parameter("bo", [1, E], bf16, isOutput=False) if has_bo else None
    out_ext = nc.declare_dram_parameter("out", [E, N], bf16, isOutput=True)

    with tile.TileContext(nc) as tc:
        with (
            tc.tile_pool(name="const", bufs=1) as cpool,
            tc.tile_pool(name="w", bufs=1) as wpool,
            tc.tile_pool(name="inT", bufs=16) as ipool,
            tc.tile_pool(name="proj", bufs=1) as ppool,
            tc.tile_pool(name="es", bufs=6) as espool,
            tc.tile_pool(name="on", bufs=13) as onpool,
            tc.tile_pool(name="nrm", bufs=2) as nrmpool,
            tc.tile_pool(name="nrm1", bufs=1) as nrm1pool,
            tc.tile_pool(name="dst", bufs=4) as dstpool,
            tc.tile_pool(name="ps_s", bufs=2, space=PSUM) as ps_s,
            tc.tile_pool(name="ps_o", bufs=2, space=PSUM) as ps_o,
            tc.tile_pool(name="ps_m", bufs=2, space=PSUM) as ps_m,
        ):
            # ---- constants -------------------------------------------------
            ones_bf = cpool.tile([1, 512], bf16, tag="ones_bf")
            nc.gpsimd.memset(ones_bf[:], 1.0)

            # ---- weights + biases -----------------------------------------
            # weight tiles declared here, DMAs emitted in first-use order below
            wq_t = wpool.tile([128, ET, S], bf16, tag="wq")
            wk_t = wpool.tile([128, ET, S], bf16, tag="wk")
            wv_t = wpool.tile([128, ET, S], bf16, tag="wv")
            wo_t = wpool.tile([128, DT, E], bf16, tag="wo")

            def load_weight(w_t, ext, ntiles):
                width = ntiles * (w_t.shape[-1])
                nc.sync.dma_start(out=w_t[:, :, :], in_=ext[:, 0:width])

            bias_tiles = {}

            def load_bias(nm, ext, width):
                if ext is not None:
                    bt = cpool.tile([1, width], bf16, tag=nm)
                    nc.sync.dma_start(out=bt[:], in_=ext[:])
                    bias_tiles[nm] = bt

            # ---- persistent activations -----------------------------------
            qpT = ppool.tile([128, DT, N], bf16, tag="qpT")   # [d, n], d-tiles = head pairs
            kpT = ppool.tile([128, DT, N], bf16, tag="kpT")
            vpa = ppool.tile([128, NT, HPC * 65], bf16, tag="vpa")  # per head: 64 V cols + ones col
            nc.gpsimd.memset(vpa[:], 1.0)  # pre-set so the ones columns survive the V copies

            # ---- phase A: projections -------------------------------------
            def load_inT(ext, quarters=False):
                ts = []
                for t in range(ET):
                    it = ipool.tile([128, N], bf16, tag="inT")
                    ts.append(it)
                nchunk = 4 if quarters else 2
                cw = N // nchunk
                for qr in range(nchunk):
                    for t in range(ET):
                        nc.sync.dma_start(
                            out=ts[t][:, qr * cw:(qr + 1) * cw],
                            in_=ext[:, t * N + qr * cw:t * N + (qr + 1) * cw],
                        )
                return ts

            def proj_group_wide(in_tiles, w_t, b_t, dest, dt, half, slot):
                # one [128, 1024] psum group: dest[:, slot, half*1024 : +1024]
                pt = ps_s.tile([128, 1024], f32, tag="s")
                n0 = half * 1024
                for et in range(ET):
                    for sub in range(2):
                        nc.tensor.matmul(
                            pt[:, sub * 512:(sub + 1) * 512],
                            w_t[:, et, dt * 128:(dt + 1) * 128],
                            in_tiles[et][:, n0 + sub * 512: n0 + (sub + 1) * 512],
                            start=(et == 0),
                            stop=(et == ET - 1 and b_t is None),
                        )
                if b_t is not None:
                    for sub in range(2):
                        nc.tensor.matmul(
                            pt[:, sub * 512:(sub + 1) * 512],
                            b_t[0:1, dt * 128:(dt + 1) * 128],
                            ones_bf[0:1, 0:512],
                            start=False, stop=True,
                        )
                nc.vector.tensor_copy(dest[:, slot, n0:n0 + 1024], pt[:, :])

            def proj_group_jit(in_tiles, w_t, b_t, dest, dt, nb, slot, on_act=False):
                # one [128, 512] psum group on tag "d": dest[:, slot, nb*512 : +512]
                pt = ps_m.tile([128, 512], f32, tag="d")
                n0 = nb * 512
                for et in range(ET):
                    nc.tensor.matmul(
                        pt[:, :],
                        w_t[:, et, dt * 128:(dt + 1) * 128],
                        in_tiles[et][:, n0:n0 + 512],
                        start=(et == 0),
                        stop=(et == ET - 1 and b_t is None),
                    )
                if b_t is not None:
                    nc.tensor.matmul(
                        pt[:, :], b_t[0:1, dt * 128:(dt + 1) * 128],
                        ones_bf[0:1, 0:512], start=False, stop=True,
                    )
                if on_act:
                    nc.scalar.copy(dest[:, slot, n0:n0 + 512], pt[:, :])
                else:
                    nc.vector.tensor_copy(dest[:, slot, n0:n0 + 512], pt[:, :])

            # V first (its input tiles release their slots for qT/kT)
            load_weight(wv_t, wv_ext, ET)
            load_bias("bv", bv_ext, S)
            v_tiles = load_inT(vT_ext, quarters=True)
            bv_t = bias_tiles.get("bv")

            def vp_group(nt):
                pt = ps_m.tile([128, 512], f32, tag="d")
                for et in range(ET):
                    nc.tensor.matmul(
                        pt[:, :],
                        v_tiles[et][:, nt * 128:(nt + 1) * 128],
                        wv_t[:, et, :],
                        start=(et == 0),
                        stop=(et == ET - 1 and bv_t is None),
                    )
                if bv_t is not None:
                    nc.tensor.matmul(
                        pt[:, :], ones_bf[0:1, 0:128], bv_t[0:1, :],
                        start=False, stop=True,
                    )
                # scatter heads into the 65-stride layout (ones col untouched);
                # phase A: the scalar engine is otherwise idle here
                dst = vpa[:, nt, :].rearrange("p (h c) -> p h c", c=65)[:, :, 0:64]
                src_ = pt[:, :].rearrange("p (h c) -> p h c", c=64)
                nc.scalar.copy(dst, src_)

            for nt in range(NT):
                vp_group(nt)

            load_weight(wk_t, wk_ext, ET)
            load_bias("bk", bk_ext, S)
            k_tiles = load_inT(kT_ext)
            load_weight(wq_t, wq_ext, ET)
            load_bias("bq", bq_ext, S)
            q_tiles = load_inT(qT_ext)
            load_weight(wo_t, wo_ext, DT)
            load_bias("bo", bo_ext, E)
            bq_t, bk_t = bias_tiles.get("bq"), bias_tiles.get("bk")
            # projections for head-pair 0 up front; later pairs are JIT filler
            # (narrow tag-"d" groups keep the "s" slots free for block 0's B)
            for nb in range(NBLK):
                proj_group_jit(k_tiles, wk_t, bk_t, kpT, 0, nb, 0, on_act=True)
            for nb in range(NBLK):
                proj_group_jit(q_tiles, wq_t, bq_t, qpT, 0, nb, 0, on_act=True)

            # ---- phases B/C/D ---------------------------------------------
            bo_t = bias_tiles.get("bo")
            on_all = [[None] * NBLK for _ in range(DT)]

            def emit_d_group(ibd, etile, on_act=False):
                    pd = ps_m.tile([128, 512], f32, tag="d")
                    for dt in range(DT):
                        nc.tensor.matmul(
                            pd[:, :],
                            wo_t[:, dt, etile * 128:(etile + 1) * 128],
                            on_all[dt][ibd][:, :],
                            start=(dt == 0),
                            stop=(dt == DT - 1 and bo_t is None),
                        )
                    if bo_t is not None:
                        nc.tensor.matmul(
                            pd[:, :],
                            bo_t[0:1, etile * 128:(etile + 1) * 128],
                            ones_bf[0:1, 0:512],
                            start=False, stop=True,
                        )
                    ds = dstpool.tile([128, 512], bf16, tag="dst")
                    if on_act:
                        nc.scalar.copy(ds[:, :], pd[:, :])
                    else:
                        nc.vector.tensor_copy(ds[:, :], pd[:, :])
                    nc.sync.dma_start(
                        out=out_ext[etile * 128:(etile + 1) * 128,
                                    ibd * 512:(ibd + 1) * 512],
                        in_=ds[:, :],
                    )

            def emit_d(ibd, on_act=False):
                # out-projection for n-block ibd; evictions alternate DVE/ACT
                for etile in range(ET):
                    emit_d_group(ibd, etile, on_act=(etile % 2 == 1))
            for hp in range(DT):
                for ib in range(NBLK):
                    i0 = ib * 512
                    o_a = ps_o.tile([65, 512], f32, tag="o")
                    o_b = ps_o.tile([65, 512], f32, tag="o")

                    def c_mms(jtc, rhs):
                        # O accumulation for consumed j-tile jtc (software-
                        # pipelined one jt behind B so the PE FIFO never
                        # stalls at the exp fence with ready B work behind it)
                        nc.tensor.matmul(
                            o_a[:, :],
                            vpa[:, jtc, (2 * hp) * 65:(2 * hp) * 65 + 65],
                            rhs[0],
                            start=(jtc == 0), stop=(jtc == NT - 1),
                        )
                        nc.tensor.matmul(
                            o_b[:, :],
                            vpa[:, jtc, (2 * hp + 1) * 65:(2 * hp + 1) * 65 + 65],
                            rhs[1],
                            start=(jtc == 0), stop=(jtc == NT - 1),
                        )

                    # process j-tiles in pairs: 4 B matmuls, 2 exps, then the
                    # previous pair's 4 C matmuls — halves the full-row-after-
                    # packed-pair LDWEIGHTS/drain tax on the PE
                    pend = []
                    for jp in range(NT // 2):
                        for jtc, rhs in pend:
                            c_mms(jtc, rhs)
                        sts = []
                        for dj in range(2):
                            jt = 2 * jp + dj
                            st = ps_s.tile([128, 1024], f32, tag="s")
                            # head A on PE rows 0-63, head B on rows 64-127
                            nc.tensor.matmul(
                                st[:, 0:512],
                                kpT[0:64, hp, jt * 128:(jt + 1) * 128],
                                qpT[0:64, hp, i0:i0 + 512],
                                start=True, stop=True,
                            )
                            nc.tensor.matmul(
                                st[:, 512:1024],
                                kpT[64:128, hp, jt * 128:(jt + 1) * 128],
                                qpT[64:128, hp, i0:i0 + 512],
                                start=True, stop=True,
                            )
                            sts.append((jt, st))
                        newpend = []
                        for jt, st in sts:
                            if jt in DVE_JT:
                                # Schraudolph exp on the vector engine
                                esi = espool.tile([128, 1024], mybir.dt.int16, tag="esI", bufs=4)
                                nc.vector.tensor_scalar(
                                    esi[:, :], st[:, :], EXP_A, EXP_B,
                                    mybir.AluOpType.mult, mybir.AluOpType.add,
                                )
                                rhs = (esi[:, 0:512].bitcast(bf16),
                                       esi[:, 512:1024].bitcast(bf16))
                            else:
                                es = espool.tile([128, 1024], bf16, tag="es")
                                nc.scalar.activation(
                                    es[:], st[:], mybir.ActivationFunctionType.Exp,
                                    scale=SCALE,
                                )
                                rhs = (es[:, 0:512], es[:, 512:1024])
                            newpend.append((jt, rhs))
                        pend = newpend
                    for jtc, rhs in pend:
                        c_mms(jtc, rhs)
                    # normalize: onorm[0:64] = O_A/Z_A, onorm[64:128] = O_B/Z_B
                    onorm = onpool.tile([128, 512], bf16, tag="onorm")
                    oc_a = nrmpool.tile([65, 512], f32, tag="oc")
                    nc.vector.tensor_copy(oc_a[:, :], o_a[:, :])  # frees psum bank
                    oc_b = nrmpool.tile([65, 512], f32, tag="oc")
                    nc.vector.tensor_copy(oc_b[:, :], o_b[:, :])
                    zr_a = nrm1pool.tile([1, 512], f32, tag="zr")
                    nc.sync.dma_start(out=zr_a[0:1, :], in_=oc_a[64:65, :])
                    zr_b = nrm1pool.tile([1, 512], f32, tag="zr")
                    nc.sync.dma_start(out=zr_b[0:1, :], in_=oc_b[64:65, :])
                    zi_a = nrm1pool.tile([1, 512], f32, tag="zi")
                    nc.vector.reciprocal_approx_fast(zi_a[0:1, :], zr_a[0:1, :])
                    zi_b = nrm1pool.tile([1, 512], f32, tag="zi")
                    nc.vector.reciprocal_approx_fast(zi_b[0:1, :], zr_b[0:1, :])
                    zb_a = nrmpool.tile([64, 512], f32, tag="zb")
                    nc.gpsimd.partition_broadcast(zb_a[:, :], zi_a[0:1, :])
                    zb_b = nrmpool.tile([64, 512], f32, tag="zb")
                    nc.gpsimd.partition_broadcast(zb_b[:, :], zi_b[0:1, :])
                    nc.vector.tensor_mul(onorm[0:64, :], oc_a[0:64, :], zb_a[:, :])
                    tmp_b = nrm1pool.tile([64, 512], bf16, tag="tmpB")
                    nc.vector.tensor_mul(tmp_b[:, :], oc_b[0:64, :], zb_b[:, :])
                    # partition shift 0-63 -> 64-127 (DMA crosses partitions)
                    nc.sync.dma_start(out=onorm[64:128, :], in_=tmp_b[:, :])
                    on_all[hp][ib] = onorm

                    if hp == DT - 1 and ib > 0:
                        # phase D one block behind (its matmuls fence on the
                        # normalize chain; keep ready B/C work ahead of them);
                        # the last of these also runs post-exp -> ACT evictions
                        emit_d(ib - 1, on_act=(ib == NBLK - 1))
                    if hp < DT - 1:
                        # JIT projections for the next head pair: PE filler
                        # that keeps the array busy while ACT drains exps.
                        proj_group_jit(k_tiles, wk_t, bk_t, kpT, hp + 1, ib,
                                       hp + 1, on_act=(ib % 2 == 0))
                        proj_group_jit(q_tiles, wq_t, bq_t, qpT, hp + 1, ib,
                                       hp + 1, on_act=(ib % 2 == 1))

                if hp == DT - 1:
                    # the final block runs after the last exp: the scalar
                    # engine is idle, so route its evictions there
                    emit_d(NBLK - 1, on_act=True)

    nc.compile()
    return nc


def _bf16c(a):
    return np.ascontiguousarray(a, dtype=np.float32).astype(BF16NP)


def kernel(q, k, v, Wq, bq, Wk, bk, Wv, bv, Wo, bo, trace=False):
    global last_exec_time_ns, last_results
    q = np.asarray(q, dtype=np.float32)
    k = np.asarray(k, dtype=np.float32)
    v = np.asarray(v, dtype=np.float32)
    Wq, Wk, Wv, Wo = (np.asarray(x, dtype=np.float32) for x in (Wq, Wk, Wv, Wo))
    bq, bk, bv, bo = (np.asarray(x, dtype=np.float32) for x in (bq, bk, bv, bo))

    has_bq, has_bk, has_bv, has_bo = (bool(np.any(x)) for x in (bq, bk, bv, bo))

    _install_ntff_shim()
    nc = _build(has_bq, has_bk, has_bv, has_bo)

    in_maps = []
    for c in range(8):
        b, g = divmod(c, 2)
        sl = slice(g * S, (g + 1) * S)
        def tile_em(x):
            # [ET*128, W] -> [128, ET*W]: partition-major tiling for fat DMA lines
            et = x.shape[0] // 128
            return np.ascontiguousarray(
                x.reshape(et, 128, x.shape[1]).transpose(1, 0, 2).reshape(128, -1))

        m = {
            "qT": _bf16c(tile_em(q[b].T)),
            "kT": _bf16c(tile_em(k[b].T)),
            "vT": _bf16c(tile_em(v[b].T)),
            "wq": _bf16c(tile_em(Wq[:, sl])),
            "wk": _bf16c(tile_em(Wk[:, sl])),
            "wv": _bf16c(tile_em(Wv[:, sl])),
            "wo": _bf16c(tile_em(Wo[sl, :])),
        }
        if has_bq:
            m["bq"] = _bf16c(bq[sl].reshape(1, S))
        if has_bk:
            m["bk"] = _bf16c(bk[sl].reshape(1, S))
        if has_bv:
            m["bv"] = _bf16c(bv[sl].reshape(1, S))
        if has_bo:
            m["bo"] = _bf16c((bo if g == 0 else np.zeros_like(bo)).reshape(1, E))
        in_maps.append(m)

    res = run_bass_kernel_spmd(nc, in_maps, core_ids=list(range(8)), trace=trace)
    last_results = res
    last_exec_time_ns = res.exec_time_ns

    out = np.empty((B, N, E), dtype=np.float32)
    for b in range(B):
        out[b] = (res.results[2 * b]["out"].astype(np.float32)
                  + res.results[2 * b + 1]["out"].astype(np.float32)).T
    return out



# revision 31
# speedup vs baseline: 1.1456x; 1.1456x over previous
"""Distributed multi-head attention kernel for 8 Trainium2 NeuronCores.

Problem: B=4, N=2048, E=1024, H=16 heads (head_dim 64), QKV + out projections.
Sharding: core c handles batch b=c//2 and head-group g=c%2 (8 heads = D-slice
of 512). QKV projections are column-sharded, the out projection is row-sharded;
the two partial outputs per batch are summed on the host during unshard.

Per-core dataflow (all matmuls bf16 with fp32 PSUM accumulation):
  A) QpT/KpT [512, 2048] and Vp [2048, 512] projections. Host pre-transposes
     q/k/v to [E, N] so the contraction dim lands on SBUF partitions.
  B) S^T[j, i] = Kp_h^T.T @ Qp_h^T per head. K=64, so head pairs are packed
     onto PE row-groups 0-63 / 64-127 (partition-base-derived tile_position).
     exp(scale*x) is fused into the PSUM->SBUF eviction on the scalar engine
     (no max-subtraction: logits are O(1) by construction).
  C) O^T_aug[65, i] accumulates Vp_aug^T @ expS^T over j-tiles, where Vp is
     augmented with a ones column so row 64 of the product is the softmax
     denominator Z.
  D) Normalize: evict O_aug to SBUF (frees the PSUM bank early), DMA the Z row
     to partition 0, reciprocal_approx_fast, gpsimd partition_broadcast, one
     multiply per head; stack head pairs (one SBUF->SBUF DMA partition shift),
     out-project, store out^T partial [1024, 2048].
"""

import sys

import numpy as np


def _ensure_paths():
    try:
        import concourse.bass  # noqa: F401
    except ImportError:
        for p in ("/opt/trn_rl_repo",):
            if p not in sys.path:
                sys.path.insert(0, p)
        import concourse.bass  # noqa: F401


_ensure_paths()

import ml_dtypes  # noqa: E402
import concourse.bass as bass  # noqa: E402
import concourse.bacc as bacc  # noqa: E402
import concourse.mybir as mybir  # noqa: E402
import concourse.tile as tile  # noqa: E402
from concourse.bass_utils import run_bass_kernel_spmd  # noqa: E402

BF16NP = ml_dtypes.bfloat16

B, N, E = 4, 2048, 1024
H, HD = 16, 64
G = 2                 # head-group (tensor-parallel) factor
S = E // G            # 512: per-core slice of the internal dim
HPC = H // G          # 8 heads per core
ET = E // 128         # 8 contraction tiles for the projections
DT = S // 128         # 4 d-tiles per core (= head pairs)
NT = N // 128         # 16 n-tiles
NBLK = N // 512       # 4 n/i blocks
SCALE = 1.0 / float(np.sqrt(HD))
# Schraudolph exp in bf16-bit space: bf16_bits(exp(s*SCALE)) ~= s*EXP_A + EXP_B
# (DVE f32->int16 convert rounds to nearest; B tuned for ~zero mean bias)
EXP_A = 128.0 * 1.4426950408889634 * SCALE
EXP_B = 127.0 * 128.0 - 7.2
# j-tiles (of 16 per block) whose exp runs on DVE via Schraudolph; the rest
# use the scalar engine's exact exp. Balances ACT vs DVE load.
DVE_JT = frozenset((1, 4, 6, 9, 11, 14))

last_exec_time_ns = None
last_results = None


def _install_ntff_shim():
    """Register the axon NTFF profile hook bass_utils wants under trace=True."""
    import types

    if "antenv.axon_hooks" in sys.modules:
        return
    mod = types.ModuleType("antenv.axon_hooks")
    _h = [None]
    mod.set_axon_ntff_profile_hook = lambda h: _h.__setitem__(0, h)
    mod.get_axon_ntff_profile_hook = lambda: _h[0]
    try:
        import antenv

        sys.modules["antenv.axon_hooks"] = mod
        antenv.axon_hooks = mod
        from trn_agent_boot.trn_boot import _ntff_profile_via_ctypes

        hook = _ntff_profile_via_ctypes("/opt/axon/libaxon_pjrt.so")
        if hook is not None:
            mod.set_axon_ntff_profile_hook(hook)
    except Exception:
        pass


def _build(has_bq, has_bk, has_bv, has_bo):
    f32 = mybir.dt.float32
    bf16 = mybir.dt.bfloat16
    PSUM = bass.MemorySpace.PSUM

    nc = bacc.Bacc("TRN2", target_bir_lowering=False, debug=False)

    # host pre-tiles all inputs partition-major: [p, et, ...] so each DMA
    # moves one contiguous multi-KB line per partition
    qT_ext = nc.declare_dram_parameter("qT", [128, ET * N], bf16, isOutput=False)
    kT_ext = nc.declare_dram_parameter("kT", [128, ET * N], bf16, isOutput=False)
    vT_ext = nc.declare_dram_parameter("vT", [128, ET * N], bf16, isOutput=False)
    wq_ext = nc.declare_dram_parameter("wq", [128, ET * S], bf16, isOutput=False)
    wk_ext = nc.declare_dram_parameter("wk", [128, ET * S], bf16, isOutput=False)
    wv_ext = nc.declare_dram_parameter("wv", [128, ET * S], bf16, isOutput=False)
    wo_ext = nc.declare_dram_parameter("wo", [128, DT * E], bf16, isOutput=False)
    bq_ext = nc.declare_dram_parameter("bq", [1, S], bf16, isOutput=False) if has_bq else None
    bk_ext = nc.declare_dram_parameter("bk", [1, S], bf16, isOutput=False) if has_bk else None
    bv_ext = nc.declare_dram_parameter("bv", [1, S], bf16, isOutput=False) if has_bv else None
    bo_ext = nc.declare_dram_parameter("bo", [1, E], bf16, isOutput=False) if has_bo else None
    out_ext = nc.declare_dram_parameter("out", [E, N], bf16, isOutput=True)

    with tile.TileContext(nc) as tc:
        with (
            tc.tile_pool(name="const", bufs=1) as cpool,
            tc.tile_pool(name="w", bufs=1) as wpool,
            tc.tile_pool(name="inT", bufs=16) as ipool,
            tc.tile_pool(name="proj", bufs=1) as ppool,
            tc.tile_pool(name="es", bufs=6) as espool,
            tc.tile_pool(name="on", bufs=13) as onpool,
            tc.tile_pool(name="nrm", bufs=2) as nrmpool,
            tc.tile_pool(name="nrm1", bufs=1) as nrm1pool,
            tc.tile_pool(name="dst", bufs=4) as dstpool,
            tc.tile_pool(name="ps_s", bufs=2, space=PSUM) as ps_s,
            tc.tile_pool(name="ps_o", bufs=2, space=PSUM) as ps_o,
            tc.tile_pool(name="ps_m", bufs=2, space=PSUM) as ps_m,
        ):
            # ---- constants -------------------------------------------------
            ones_bf = cpool.tile([1, 512], bf16, tag="ones_bf")
            nc.gpsimd.memset(ones_bf[:], 1.0)

            # ---- weights + biases -----------------------------------------
            # weight tiles declared here, DMAs emitted in first-use order below
            wq_t = wpool.tile([128, ET, S], bf16, tag="wq")
            wk_t = wpool.tile([128, ET, S], bf16, tag="wk")
            wv_t = wpool.tile([128, ET, S], bf16, tag="wv")
            wo_t = wpool.tile([128, DT, E], bf16, tag="wo")

            def load_weight(w_t, ext, ntiles):
                width = ntiles * (w_t.shape[-1])
                nc.sync.dma_start(out=w_t[:, :, :], in_=ext[:, 0:width])

            bias_tiles = {}

            def load_bias(nm, ext, width):
                if ext is not None:
                    bt = cpool.tile([1, width], bf16, tag=nm)
                    nc.sync.dma_start(out=bt[:], in_=ext[:])
                    bias_tiles[nm] = bt

            # ---- persistent activations -----------------------------------
            qpT = ppool.tile([128, DT, N], bf16, tag="qpT")   # [d, n], d-tiles = head pairs
            kpT = ppool.tile([128, DT, N], bf16, tag="kpT")
            vpa = ppool.tile([128, NT, HPC * 65], bf16, tag="vpa")  # per head: 64 V cols + ones col
            nc.gpsimd.memset(vpa[:], 1.0)  # pre-set so the ones columns survive the V copies

            # ---- phase A: projections -------------------------------------
            def load_inT(ext, quarters=False):
                ts = []
                for t in range(ET):
                    it = ipool.tile([128, N], bf16, tag="inT")
                    ts.append(it)
                nchunk = 4 if quarters else 2
                cw = N // nchunk
                for qr in range(nchunk):
                    for t in range(ET):
                        nc.sync.dma_start(
                            out=ts[t][:, qr * cw:(qr + 1) * cw],
                            in_=ext[:, t * N + qr * cw:t * N + (qr + 1) * cw],
                        )
                return ts

            def proj_group_wide(in_tiles, w_t, b_t, dest, dt, half, slot):
                # one [128, 1024] psum group: dest[:, slot, half*1024 : +1024]
                pt = ps_s.tile([128, 1024], f32, tag="s")
                n0 = half * 1024
                for et in range(ET):
                    for sub in range(2):
                        nc.tensor.matmul(
                            pt[:, sub * 512:(sub + 1) * 512],
                            w_t[:, et, dt * 128:(dt + 1) * 128],
                            in_tiles[et][:, n0 + sub * 512: n0 + (sub + 1) * 512],
                            start=(et == 0),
                            stop=(et == ET - 1 and b_t is None),
                        )
                if b_t is not None:
                    for sub in range(2):
                        nc.tensor.matmul(
                            pt[:, sub * 512:(sub + 1) * 512],
                            b_t[0:1, dt * 128:(dt + 1) * 128],
                            ones_bf[0:1, 0:512],
                            start=False, stop=True,
                        )
                nc.vector.tensor_copy(dest[:, slot, n0:n0 + 1024], pt[:, :])

            def proj_group_jit(in_tiles, w_t, b_t, dest, dt, nb, slot, on_act=False):
                # one [128, 512] psum group on tag "d": dest[:, slot, nb*512 : +512]
                pt = ps_m.tile([128, 512], f32, tag="d")
                n0 = nb * 512
                for et in range(ET):
                    nc.tensor.matmul(
                        pt[:, :],
                        w_t[:, et, dt * 128:(dt + 1) * 128],
                        in_tiles[et][:, n0:n0 + 512],
                        start=(et == 0),
                        stop=(et == ET - 1 and b_t is None),
                    )
                if b_t is not None:
                    nc.tensor.matmul(
                        pt[:, :], b_t[0:1, dt * 128:(dt + 1) * 128],
                        ones_bf[0:1, 0:512], start=False, stop=True,
                    )
                if on_act:
                    nc.scalar.copy(dest[:, slot, n0:n0 + 512], pt[:, :])
                else:
                    nc.vector.tensor_copy(dest[:, slot, n0:n0 + 512], pt[:, :])

            # V first (its input tiles release their slots for qT/kT)
            load_weight(wv_t, wv_ext, ET)
            load_bias("bv", bv_ext, S)
            v_tiles = load_inT(vT_ext, quarters=True)
            bv_t = bias_tiles.get("bv")

            def vp_group(nt):
                pt = ps_m.tile([128, 512], f32, tag="d")
                for et in range(ET):
                    nc.tensor.matmul(
                        pt[:, :],
                        v_tiles[et][:, nt * 128:(nt + 1) * 128],
                        wv_t[:, et, :],
                        start=(et == 0),
                        stop=(et == ET - 1 and bv_t is None),
                    )
                if bv_t is not None:
                    nc.tensor.matmul(
                        pt[:, :], ones_bf[0:1, 0:128], bv_t[0:1, :],
                        start=False, stop=True,
                    )
                # scatter heads into the 65-stride layout (ones col untouched);
                # phase A: the scalar engine is otherwise idle here
                dst = vpa[:, nt, :].rearrange("p (h c) -> p h c", c=65)[:, :, 0:64]
                src_ = pt[:, :].rearrange("p (h c) -> p h c", c=64)
                nc.scalar.copy(dst, src_)

            for nt in range(NT):
                vp_group(nt)

            load_weight(wk_t, wk_ext, ET)
            load_bias("bk", bk_ext, S)
            k_tiles = load_inT(kT_ext)
            load_weight(wq_t, wq_ext, ET)
            load_bias("bq", bq_ext, S)
            q_tiles = load_inT(qT_ext)
            load_weight(wo_t, wo_ext, DT)
            load_bias("bo", bo_ext, E)
            bq_t, bk_t = bias_tiles.get("bq"), bias_tiles.get("bk")
            # projections for head-pair 0 up front; later pairs are JIT filler
            # (narrow tag-"d" groups keep the "s" slots free for block 0's B)
            for nb in range(NBLK):
                proj_group_jit(k_tiles, wk_t, bk_t, kpT, 0, nb, 0, on_act=True)
            for nb in range(NBLK):
                proj_group_jit(q_tiles, wq_t, bq_t, qpT, 0, nb, 0, on_act=True)

            # ---- phases B/C/D ---------------------------------------------
            bo_t = bias_tiles.get("bo")
            on_all = [[None] * NBLK for _ in range(DT)]

            def emit_d_group(ibd, etile, on_act=False):
                    pd = ps_m.tile([128, 512], f32, tag="d")
                    for dt in range(DT):
                        nc.tensor.matmul(
                            pd[:, :],
                            wo_t[:, dt, etile * 128:(etile + 1) * 128],
                            on_all[dt][ibd][:, :],
                            start=(dt == 0),
                            stop=(dt == DT - 1 and bo_t is None),
                        )
                    if bo_t is not None:
                        nc.tensor.matmul(
                            pd[:, :],
                            bo_t[0:1, etile * 128:(etile + 1) * 128],
                            ones_bf[0:1, 0:512],
                            start=False, stop=True,
                        )
                    ds = dstpool.tile([128, 512], bf16, tag="dst")
                    if on_act:
                        nc.scalar.copy(ds[:, :], pd[:, :])
                    else:
                        nc.vector.tensor_copy(ds[:, :], pd[:, :])
                    nc.sync.dma_start(
                        out=out_ext[etile * 128:(etile + 1) * 128,
                                    ibd * 512:(ibd + 1) * 512],
                        in_=ds[:, :],
                    )

            def emit_d(ibd, on_act=False):
                # out-projection for n-block ibd; evictions alternate DVE/ACT
                for etile in range(ET):
                    emit_d_group(ibd, etile, on_act=(etile % 2 == 1))
            for hp in range(DT):
                for ib in range(NBLK):
                    i0 = ib * 512
                    o_a = ps_o.tile([65, 512], f32, tag="o")
                    o_b = ps_o.tile([65, 512], f32, tag="o")

                    def c_mms(jtc, rhs):
                        # O accumulation for consumed j-tile jtc (software-
                        # pipelined one jt behind B so the PE FIFO never
                        # stalls at the exp fence with ready B work behind it)
                        nc.tensor.matmul(
                            o_a[:, :],
                            vpa[:, jtc, (2 * hp) * 65:(2 * hp) * 65 + 65],
                            rhs[0],
                            start=(jtc == 0), stop=(jtc == NT - 1),
                        )
                        nc.tensor.matmul(
                            o_b[:, :],
                            vpa[:, jtc, (2 * hp + 1) * 65:(2 * hp + 1) * 65 + 65],
                            rhs[1],
                            start=(jtc == 0), stop=(jtc == NT - 1),
                        )

                    # process j-tiles in pairs: 4 B matmuls, 2 exps, then the
                    # previous pair's 4 C matmuls — halves the full-row-after-
                    # packed-pair LDWEIGHTS/drain tax on the PE
                    pend = []
                    for jp in range(NT // 2):
                        sts = []
                        for dj in range(2):
                            jt = 2 * jp + dj
                            st = ps_s.tile([128, 1024], f32, tag="s")
                            # head A on PE rows 0-63, head B on rows 64-127
                            nc.tensor.matmul(
                                st[:, 0:512],
                                kpT[0:64, hp, jt * 128:(jt + 1) * 128],
                                qpT[0:64, hp, i0:i0 + 512],
                                start=True, stop=True,
                            )
                            nc.tensor.matmul(
                                st[:, 512:1024],
                                kpT[64:128, hp, jt * 128:(jt + 1) * 128],
                                qpT[64:128, hp, i0:i0 + 512],
                                start=True, stop=True,
                            )
                            sts.append((jt, st))
                        newpend = []
                        for jt, st in sts:
                            if jt in DVE_JT:
                                # Schraudolph exp on the vector engine
                                esi = espool.tile([128, 1024], mybir.dt.int16, tag="esI", bufs=4)
                                nc.vector.tensor_scalar(
                                    esi[:, :], st[:, :], EXP_A, EXP_B,
                                    mybir.AluOpType.mult, mybir.AluOpType.add,
                                )
                                rhs = (esi[:, 0:512].bitcast(bf16),
                                       esi[:, 512:1024].bitcast(bf16))
                            else:
                                es = espool.tile([128, 1024], bf16, tag="es")
                                nc.scalar.activation(
                                    es[:], st[:], mybir.ActivationFunctionType.Exp,
                                    scale=SCALE,
                                )
                                rhs = (es[:, 0:512], es[:, 512:1024])
                            newpend.append((jt, rhs))
                        for jtc, rhs in pend:
                            c_mms(jtc, rhs)
                        pend = newpend
                    for jtc, rhs in pend:
                        c_mms(jtc, rhs)
                    # normalize: onorm[0:64] = O_A/Z_A, onorm[64:128] = O_B/Z_B
                    onorm = onpool.tile([128, 512], bf16, tag="onorm")
                    oc_a = nrmpool.tile([65, 512], f32, tag="oc")
                    nc.vector.tensor_copy(oc_a[:, :], o_a[:, :])  # frees psum bank
                    oc_b = nrmpool.tile([65, 512], f32, tag="oc")
                    nc.vector.tensor_copy(oc_b[:, :], o_b[:, :])
                    zr_a = nrm1pool.tile([1, 512], f32, tag="zr")
                    nc.sync.dma_start(out=zr_a[0:1, :], in_=oc_a[64:65, :])
                    zr_b = nrm1pool.tile([1, 512], f32, tag="zr")
                    nc.sync.dma_start(out=zr_b[0:1, :], in_=oc_b[64:65, :])
                    zi_a = nrm1pool.tile([1, 512], f32, tag="zi")
                    nc.vector.reciprocal_approx_fast(zi_a[0:1, :], zr_a[0:1, :])
                    zi_b = nrm1pool.tile([1, 512], f32, tag="zi")
                    nc.vector.reciprocal_approx_fast(zi_b[0:1, :], zr_b[0:1, :])
                    zb_a = nrmpool.tile([64, 512], f32, tag="zb")
                    nc.gpsimd.partition_broadcast(zb_a[:, :], zi_a[0:1, :])
                    zb_b = nrmpool.tile([64, 512], f32, tag="zb")
                    nc.gpsimd.partition_broadcast(zb_b[:, :], zi_b[0:1, :])
                    nc.vector.tensor_mul(onorm[0:64, :], oc_a[0:64, :], zb_a[:, :])
                    tmp_b = nrm1pool.tile([64, 512], bf16, tag="tmpB")
                    nc.vector.tensor_mul(tmp_b[:, :], oc_b[0:64, :], zb_b[:, :])
                    # partition shift 0-63 -> 64-127 (DMA crosses partitions)
                    nc.sync.dma_start(out=onorm[64:128, :], in_=tmp_b[:, :])
                    on_all[hp][ib] = onorm

                    if hp == DT - 1 and ib > 0:
                        # phase D one block behind (its matmuls fence on the
                        # normalize chain; keep ready B/C work ahead of them);
                        # the last of these also runs post-exp -> ACT evictions
                        emit_d(ib - 1, on_act=(ib == NBLK - 1))
                    if hp < DT - 1:
                        # JIT projections for the next head pair: PE filler
                        # that keeps the array busy while ACT drains exps.
                        proj_group_jit(k_tiles, wk_t, bk_t, kpT, hp + 1, ib,
                                       hp + 1, on_act=(ib % 2 == 0))
                        proj_group_jit(q_tiles, wq_t, bq_t, qpT, hp + 1, ib,
                                       hp + 1, on_act=(ib % 2 == 1))

                if hp == DT - 1:
                    # the final block runs after the last exp: the scalar
                    # engine is idle, so route its evictions there
                    emit_d(NBLK - 1, on_act=True)

    nc.compile()
    return nc


def _bf16c(a):
    return np.ascontiguousarray(a, dtype=np.float32).astype(BF16NP)


def kernel(q, k, v, Wq, bq, Wk, bk, Wv, bv, Wo, bo, trace=False):
    global last_exec_time_ns, last_results
    q = np.asarray(q, dtype=np.float32)
    k = np.asarray(k, dtype=np.float32)
    v = np.asarray(v, dtype=np.float32)
    Wq, Wk, Wv, Wo = (np.asarray(x, dtype=np.float32) for x in (Wq, Wk, Wv, Wo))
    bq, bk, bv, bo = (np.asarray(x, dtype=np.float32) for x in (bq, bk, bv, bo))

    has_bq, has_bk, has_bv, has_bo = (bool(np.any(x)) for x in (bq, bk, bv, bo))

    _install_ntff_shim()
    nc = _build(has_bq, has_bk, has_bv, has_bo)

    in_maps = []
    for c in range(8):
        b, g = divmod(c, 2)
        sl = slice(g * S, (g + 1) * S)
        def tile_em(x):
            # [ET*128, W] -> [128, ET*W]: partition-major tiling for fat DMA lines
            et = x.shape[0] // 128
            return np.ascontiguousarray(
                x.reshape(et, 128, x.shape[1]).transpose(1, 0, 2).reshape(128, -1))

        m = {
            "qT": _bf16c(tile_em(q[b].T)),
            "kT": _bf16c(tile_em(k[b].T)),
            "vT": _bf16c(tile_em(v[b].T)),
            "wq": _bf16c(tile_em(Wq[:, sl])),
            "wk": _bf16c(tile_em(Wk[:, sl])),
            "wv": _bf16c(tile_em(Wv[:, sl])),
            "wo": _bf16c(tile_em(Wo[sl, :])),
        }
        if has_bq:
            m["bq"] = _bf16c(bq[sl].reshape(1, S))
        if has_bk:
            m["bk"] = _bf16c(bk[sl].reshape(1, S))
        if has_bv:
            m["bv"] = _bf16c(bv[sl].reshape(1, S))
        if has_bo:
            m["bo"] = _bf16c((bo if g == 0 else np.zeros_like(bo)).reshape(1, E))
        in_maps.append(m)

    res = run_bass_kernel_spmd(nc, in_maps, core_ids=list(range(8)), trace=trace)
    last_results = res
    last_exec_time_ns = res.exec_time_ns

    out = np.empty((B, N, E), dtype=np.float32)
    for b in range(B):
        out[b] = (res.results[2 * b]["out"].astype(np.float32)
                  + res.results[2 * b + 1]["out"].astype(np.float32)).T
    return out



# revision 32
# speedup vs baseline: 1.1469x; 1.0011x over previous
"""Distributed multi-head attention kernel for 8 Trainium2 NeuronCores.

Problem: B=4, N=2048, E=1024, H=16 heads (head_dim 64), QKV + out projections.
Sharding: core c handles batch b=c//2 and head-group g=c%2 (8 heads = D-slice
of 512). QKV projections are column-sharded, the out projection is row-sharded;
the two partial outputs per batch are summed on the host during unshard.

Per-core dataflow (all matmuls bf16 with fp32 PSUM accumulation):
  A) QpT/KpT [512, 2048] and Vp [2048, 512] projections. Host pre-transposes
     q/k/v to [E, N] so the contraction dim lands on SBUF partitions.
  B) S^T[j, i] = Kp_h^T.T @ Qp_h^T per head. K=64, so head pairs are packed
     onto PE row-groups 0-63 / 64-127 (partition-base-derived tile_position).
     exp(scale*x) is fused into the PSUM->SBUF eviction on the scalar engine
     (no max-subtraction: logits are O(1) by construction).
  C) O^T_aug[65, i] accumulates Vp_aug^T @ expS^T over j-tiles, where Vp is
     augmented with a ones column so row 64 of the product is the softmax
     denominator Z.
  D) Normalize: evict O_aug to SBUF (frees the PSUM bank early), DMA the Z row
     to partition 0, reciprocal_approx_fast, gpsimd partition_broadcast, one
     multiply per head; stack head pairs (one SBUF->SBUF DMA partition shift),
     out-project, store out^T partial [1024, 2048].
"""

import sys

import numpy as np


def _ensure_paths():
    try:
        import concourse.bass  # noqa: F401
    except ImportError:
        for p in ("/opt/trn_rl_repo",):
            if p not in sys.path:
                sys.path.insert(0, p)
        import concourse.bass  # noqa: F401


_ensure_paths()

import ml_dtypes  # noqa: E402
import concourse.bass as bass  # noqa: E402
import concourse.bacc as bacc  # noqa: E402
import concourse.mybir as mybir  # noqa: E402
import concourse.tile as tile  # noqa: E402
from concourse.bass_utils import run_bass_kernel_spmd  # noqa: E402

BF16NP = ml_dtypes.bfloat16

B, N, E = 4, 2048, 1024
H, HD = 16, 64
G = 2                 # head-group (tensor-parallel) factor
S = E // G            # 512: per-core slice of the internal dim
HPC = H // G          # 8 heads per core
ET = E // 128         # 8 contraction tiles for the projections
DT = S // 128         # 4 d-tiles per core (= head pairs)
NT = N // 128         # 16 n-tiles
NBLK = N // 512       # 4 n/i blocks
SCALE = 1.0 / float(np.sqrt(HD))
# Schraudolph exp in bf16-bit space: bf16_bits(exp(s*SCALE)) ~= s*EXP_A + EXP_B
# (DVE f32->int16 convert rounds to nearest; B tuned for ~zero mean bias)
EXP_A = 128.0 * 1.4426950408889634 * SCALE
EXP_B = 127.0 * 128.0 - 7.2
# j-tiles (of 16 per block) whose exp runs on DVE via Schraudolph; the rest
# use the scalar engine's exact exp. Balances ACT vs DVE load.
DVE_JT = frozenset((1, 4, 6, 9, 11, 14))

last_exec_time_ns = None
last_results = None


def _install_ntff_shim():
    """Register the axon NTFF profile hook bass_utils wants under trace=True."""
    import types

    if "antenv.axon_hooks" in sys.modules:
        return
    mod = types.ModuleType("antenv.axon_hooks")
    _h = [None]
    mod.set_axon_ntff_profile_hook = lambda h: _h.__setitem__(0, h)
    mod.get_axon_ntff_profile_hook = lambda: _h[0]
    try:
        import antenv

        sys.modules["antenv.axon_hooks"] = mod
        antenv.axon_hooks = mod
        from trn_agent_boot.trn_boot import _ntff_profile_via_ctypes

        hook = _ntff_profile_via_ctypes("/opt/axon/libaxon_pjrt.so")
        if hook is not None:
            mod.set_axon_ntff_profile_hook(hook)
    except Exception:
        pass


def _build(has_bq, has_bk, has_bv, has_bo):
    f32 = mybir.dt.float32
    bf16 = mybir.dt.bfloat16
    PSUM = bass.MemorySpace.PSUM

    nc = bacc.Bacc("TRN2", target_bir_lowering=False, debug=False)

    # host pre-tiles all inputs partition-major: [p, et, ...] so each DMA
    # moves one contiguous multi-KB line per partition
    qT_ext = nc.declare_dram_parameter("qT", [128, ET * N], bf16, isOutput=False)
    kT_ext = nc.declare_dram_parameter("kT", [128, ET * N], bf16, isOutput=False)
    vT_ext = nc.declare_dram_parameter("vT", [128, ET * N], bf16, isOutput=False)
    wq_ext = nc.declare_dram_parameter("wq", [128, ET * S], bf16, isOutput=False)
    wk_ext = nc.declare_dram_parameter("wk", [128, ET * S], bf16, isOutput=False)
    wv_ext = nc.declare_dram_parameter("wv", [128, ET * S], bf16, isOutput=False)
    wo_ext = nc.declare_dram_parameter("wo", [128, DT * E], bf16, isOutput=False)
    bq_ext = nc.declare_dram_parameter("bq", [1, S], bf16, isOutput=False) if has_bq else None
    bk_ext = nc.declare_dram_parameter("bk", [1, S], bf16, isOutput=False) if has_bk else None
    bv_ext = nc.declare_dram_parameter("bv", [1, S], bf16, isOutput=False) if has_bv else None
    bo_ext = nc.declare_dram_parameter("bo", [1, E], bf16, isOutput=False) if has_bo else None
    out_ext = nc.declare_dram_parameter("out", [E, N], f32, isOutput=True)

    with tile.TileContext(nc) as tc:
        with (
            tc.tile_pool(name="const", bufs=1) as cpool,
            tc.tile_pool(name="w", bufs=1) as wpool,
            tc.tile_pool(name="inT", bufs=16) as ipool,
            tc.tile_pool(name="proj", bufs=1) as ppool,
            tc.tile_pool(name="es", bufs=6) as espool,
            tc.tile_pool(name="on", bufs=13) as onpool,
            tc.tile_pool(name="nrm", bufs=2) as nrmpool,
            tc.tile_pool(name="nrm1", bufs=1) as nrm1pool,
            tc.tile_pool(name="dst", bufs=4) as dstpool,
            tc.tile_pool(name="ps_s", bufs=2, space=PSUM) as ps_s,
            tc.tile_pool(name="ps_o", bufs=2, space=PSUM) as ps_o,
            tc.tile_pool(name="ps_m", bufs=2, space=PSUM) as ps_m,
        ):
            # ---- constants -------------------------------------------------
            ones_bf = cpool.tile([1, 512], bf16, tag="ones_bf")
            nc.gpsimd.memset(ones_bf[:], 1.0)

            # ---- weights + biases -----------------------------------------
            # weight tiles declared here, DMAs emitted in first-use order below
            wq_t = wpool.tile([128, ET, S], bf16, tag="wq")
            wk_t = wpool.tile([128, ET, S], bf16, tag="wk")
            wv_t = wpool.tile([128, ET, S], bf16, tag="wv")
            wo_t = wpool.tile([128, DT, E], bf16, tag="wo")

            def load_weight(w_t, ext, ntiles):
                width = ntiles * (w_t.shape[-1])
                nc.sync.dma_start(out=w_t[:, :, :], in_=ext[:, 0:width])

            bias_tiles = {}

            def load_bias(nm, ext, width):
                if ext is not None:
                    bt = cpool.tile([1, width], bf16, tag=nm)
                    nc.sync.dma_start(out=bt[:], in_=ext[:])
                    bias_tiles[nm] = bt

            # ---- persistent activations -----------------------------------
            qpT = ppool.tile([128, DT, N], bf16, tag="qpT")   # [d, n], d-tiles = head pairs
            kpT = ppool.tile([128, DT, N], bf16, tag="kpT")
            vpa = ppool.tile([128, NT, HPC * 65], bf16, tag="vpa")  # per head: 64 V cols + ones col
            nc.gpsimd.memset(vpa[:], 1.0)  # pre-set so the ones columns survive the V copies

            # ---- phase A: projections -------------------------------------
            def load_inT(ext, quarters=False):
                ts = []
                for t in range(ET):
                    it = ipool.tile([128, N], bf16, tag="inT")
                    ts.append(it)
                nchunk = 4 if quarters else 2
                cw = N // nchunk
                for qr in range(nchunk):
                    for t in range(ET):
                        nc.sync.dma_start(
                            out=ts[t][:, qr * cw:(qr + 1) * cw],
                            in_=ext[:, t * N + qr * cw:t * N + (qr + 1) * cw],
                        )
                return ts

            def proj_group_wide(in_tiles, w_t, b_t, dest, dt, half, slot):
                # one [128, 1024] psum group: dest[:, slot, half*1024 : +1024]
                pt = ps_s.tile([128, 1024], f32, tag="s")
                n0 = half * 1024
                for et in range(ET):
                    for sub in range(2):
                        nc.tensor.matmul(
                            pt[:, sub * 512:(sub + 1) * 512],
                            w_t[:, et, dt * 128:(dt + 1) * 128],
                            in_tiles[et][:, n0 + sub * 512: n0 + (sub + 1) * 512],
                            start=(et == 0),
                            stop=(et == ET - 1 and b_t is None),
                        )
                if b_t is not None:
                    for sub in range(2):
                        nc.tensor.matmul(
                            pt[:, sub * 512:(sub + 1) * 512],
                            b_t[0:1, dt * 128:(dt + 1) * 128],
                            ones_bf[0:1, 0:512],
                            start=False, stop=True,
                        )
                nc.vector.tensor_copy(dest[:, slot, n0:n0 + 1024], pt[:, :])

            def proj_group_jit(in_tiles, w_t, b_t, dest, dt, nb, slot, on_act=False):
                # one [128, 512] psum group on tag "d": dest[:, slot, nb*512 : +512]
                pt = ps_m.tile([128, 512], f32, tag="d")
                n0 = nb * 512
                for et in range(ET):
                    nc.tensor.matmul(
                        pt[:, :],
                        w_t[:, et, dt * 128:(dt + 1) * 128],
                        in_tiles[et][:, n0:n0 + 512],
                        start=(et == 0),
                        stop=(et == ET - 1 and b_t is None),
                    )
                if b_t is not None:
                    nc.tensor.matmul(
                        pt[:, :], b_t[0:1, dt * 128:(dt + 1) * 128],
                        ones_bf[0:1, 0:512], start=False, stop=True,
                    )
                if on_act:
                    nc.scalar.copy(dest[:, slot, n0:n0 + 512], pt[:, :])
                else:
                    nc.vector.tensor_copy(dest[:, slot, n0:n0 + 512], pt[:, :])

            # V first (its input tiles release their slots for qT/kT)
            load_weight(wv_t, wv_ext, ET)
            load_bias("bv", bv_ext, S)
            v_tiles = load_inT(vT_ext, quarters=True)
            bv_t = bias_tiles.get("bv")

            def vp_group(nt):
                pt = ps_m.tile([128, 512], f32, tag="d")
                for et in range(ET):
                    nc.tensor.matmul(
                        pt[:, :],
                        v_tiles[et][:, nt * 128:(nt + 1) * 128],
                        wv_t[:, et, :],
                        start=(et == 0),
                        stop=(et == ET - 1 and bv_t is None),
                    )
                if bv_t is not None:
                    nc.tensor.matmul(
                        pt[:, :], ones_bf[0:1, 0:128], bv_t[0:1, :],
                        start=False, stop=True,
                    )
                # scatter heads into the 65-stride layout (ones col untouched);
                # phase A: the scalar engine is otherwise idle here
                dst = vpa[:, nt, :].rearrange("p (h c) -> p h c", c=65)[:, :, 0:64]
                src_ = pt[:, :].rearrange("p (h c) -> p h c", c=64)
                nc.scalar.copy(dst, src_)

            for nt in range(NT):
                vp_group(nt)

            load_weight(wk_t, wk_ext, ET)
            load_bias("bk", bk_ext, S)
            k_tiles = load_inT(kT_ext)
            load_weight(wq_t, wq_ext, ET)
            load_bias("bq", bq_ext, S)
            q_tiles = load_inT(qT_ext)
            load_weight(wo_t, wo_ext, DT)
            load_bias("bo", bo_ext, E)
            bq_t, bk_t = bias_tiles.get("bq"), bias_tiles.get("bk")
            # projections for head-pair 0 up front; later pairs are JIT filler
            # (narrow tag-"d" groups keep the "s" slots free for block 0's B)
            for nb in range(NBLK):
                proj_group_jit(k_tiles, wk_t, bk_t, kpT, 0, nb, 0, on_act=True)
            for nb in range(NBLK):
                proj_group_jit(q_tiles, wq_t, bq_t, qpT, 0, nb, 0, on_act=True)

            # ---- phases B/C/D ---------------------------------------------
            bo_t = bias_tiles.get("bo")
            on_all = [[None] * NBLK for _ in range(DT)]

            def emit_d_group(ibd, etile, on_act=False):
                    pd = ps_m.tile([128, 512], f32, tag="d")
                    for dt in range(DT):
                        nc.tensor.matmul(
                            pd[:, :],
                            wo_t[:, dt, etile * 128:(etile + 1) * 128],
                            on_all[dt][ibd][:, :],
                            start=(dt == 0),
                            stop=(dt == DT - 1 and bo_t is None),
                        )
                    if bo_t is not None:
                        nc.tensor.matmul(
                            pd[:, :],
                            bo_t[0:1, etile * 128:(etile + 1) * 128],
                            ones_bf[0:1, 0:512],
                            start=False, stop=True,
                        )
                    ds = dstpool.tile([128, 512], f32, tag="dst")
                    if on_act:
                        nc.scalar.copy(ds[:, :], pd[:, :])
                    else:
                        nc.vector.tensor_copy(ds[:, :], pd[:, :])
                    nc.sync.dma_start(
                        out=out_ext[etile * 128:(etile + 1) * 128,
                                    ibd * 512:(ibd + 1) * 512],
                        in_=ds[:, :],
                    )

            def emit_d(ibd, on_act=False):
                # out-projection for n-block ibd; evictions alternate DVE/ACT
                for etile in range(ET):
                    emit_d_group(ibd, etile, on_act=(etile % 2 == 1))
            for hp in range(DT):
                for ib in range(NBLK):
                    i0 = ib * 512
                    o_a = ps_o.tile([65, 512], f32, tag="o")
                    o_b = ps_o.tile([65, 512], f32, tag="o")

                    def c_mms(jtc, rhs):
                        # O accumulation for consumed j-tile jtc (software-
                        # pipelined one jt behind B so the PE FIFO never
                        # stalls at the exp fence with ready B work behind it)
                        nc.tensor.matmul(
                            o_a[:, :],
                            vpa[:, jtc, (2 * hp) * 65:(2 * hp) * 65 + 65],
                            rhs[0],
                            start=(jtc == 0), stop=(jtc == NT - 1),
                        )
                        nc.tensor.matmul(
                            o_b[:, :],
                            vpa[:, jtc, (2 * hp + 1) * 65:(2 * hp + 1) * 65 + 65],
                            rhs[1],
                            start=(jtc == 0), stop=(jtc == NT - 1),
                        )

                    # process j-tiles in pairs: 4 B matmuls, 2 exps, then the
                    # previous pair's 4 C matmuls — halves the full-row-after-
                    # packed-pair LDWEIGHTS/drain tax on the PE
                    pend = []
                    for jp in range(NT // 2):
                        sts = []
                        for dj in range(2):
                            jt = 2 * jp + dj
                            st = ps_s.tile([128, 1024], f32, tag="s")
                            # head A on PE rows 0-63, head B on rows 64-127
                            nc.tensor.matmul(
                                st[:, 0:512],
                                kpT[0:64, hp, jt * 128:(jt + 1) * 128],
                                qpT[0:64, hp, i0:i0 + 512],
                                start=True, stop=True,
                            )
                            nc.tensor.matmul(
                                st[:, 512:1024],
                                kpT[64:128, hp, jt * 128:(jt + 1) * 128],
                                qpT[64:128, hp, i0:i0 + 512],
                                start=True, stop=True,
                            )
                            sts.append((jt, st))
                        newpend = []
                        for jt, st in sts:
                            if jt in DVE_JT:
                                # Schraudolph exp on the vector engine
                                esi = espool.tile([128, 1024], mybir.dt.int16, tag="esI", bufs=4)
                                nc.vector.tensor_scalar(
                                    esi[:, :], st[:, :], EXP_A, EXP_B,
                                    mybir.AluOpType.mult, mybir.AluOpType.add,
                                )
                                rhs = (esi[:, 0:512].bitcast(bf16),
                                       esi[:, 512:1024].bitcast(bf16))
                            else:
                                es = espool.tile([128, 1024], bf16, tag="es")
                                nc.scalar.activation(
                                    es[:], st[:], mybir.ActivationFunctionType.Exp,
                                    scale=SCALE,
                                )
                                rhs = (es[:, 0:512], es[:, 512:1024])
                            newpend.append((jt, rhs))
                        for jtc, rhs in pend:
                            c_mms(jtc, rhs)
                        pend = newpend
                    for jtc, rhs in pend:
                        c_mms(jtc, rhs)
                    # normalize: onorm[0:64] = O_A/Z_A, onorm[64:128] = O_B/Z_B
                    onorm = onpool.tile([128, 512], bf16, tag="onorm")
                    oc_a = nrmpool.tile([65, 512], f32, tag="oc")
                    nc.vector.tensor_copy(oc_a[:, :], o_a[:, :])  # frees psum bank
                    oc_b = nrmpool.tile([65, 512], f32, tag="oc")
                    nc.vector.tensor_copy(oc_b[:, :], o_b[:, :])
                    zr_a = nrm1pool.tile([1, 512], f32, tag="zr")
                    nc.sync.dma_start(out=zr_a[0:1, :], in_=oc_a[64:65, :])
                    zr_b = nrm1pool.tile([1, 512], f32, tag="zr")
                    nc.sync.dma_start(out=zr_b[0:1, :], in_=oc_b[64:65, :])
                    zi_a = nrm1pool.tile([1, 512], f32, tag="zi")
                    nc.vector.reciprocal_approx_fast(zi_a[0:1, :], zr_a[0:1, :])
                    zi_b = nrm1pool.tile([1, 512], f32, tag="zi")
                    nc.vector.reciprocal_approx_fast(zi_b[0:1, :], zr_b[0:1, :])
                    zb_a = nrmpool.tile([64, 512], f32, tag="zb")
                    nc.gpsimd.partition_broadcast(zb_a[:, :], zi_a[0:1, :])
                    zb_b = nrmpool.tile([64, 512], f32, tag="zb")
                    nc.gpsimd.partition_broadcast(zb_b[:, :], zi_b[0:1, :])
                    nc.vector.tensor_mul(onorm[0:64, :], oc_a[0:64, :], zb_a[:, :])
                    tmp_b = nrm1pool.tile([64, 512], bf16, tag="tmpB")
                    nc.vector.tensor_mul(tmp_b[:, :], oc_b[0:64, :], zb_b[:, :])
                    # partition shift 0-63 -> 64-127 (DMA crosses partitions)
                    nc.sync.dma_start(out=onorm[64:128, :], in_=tmp_b[:, :])
                    on_all[hp][ib] = onorm

                    if hp == DT - 1 and ib > 0:
                        # phase D one block behind (its matmuls fence on the
                        # normalize chain; keep ready B/C work ahead of them);
                        # the last of these also runs post-exp -> ACT evictions
                        emit_d(ib - 1, on_act=(ib == NBLK - 1))
                    if hp < DT - 1:
                        # JIT projections for the next head pair: PE filler
                        # that keeps the array busy while ACT drains exps.
                        proj_group_jit(k_tiles, wk_t, bk_t, kpT, hp + 1, ib,
                                       hp + 1, on_act=(ib % 2 == 0))
                        proj_group_jit(q_tiles, wq_t, bq_t, qpT, hp + 1, ib,
                                       hp + 1, on_act=(ib % 2 == 1))

                if hp == DT - 1:
                    # the final block runs after the last exp: the scalar
                    # engine is idle, so route its evictions there
                    emit_d(NBLK - 1, on_act=True)

    nc.compile()
    return nc


def _bf16c(a):
    return np.ascontiguousarray(a, dtype=np.float32).astype(BF16NP)


def kernel(q, k, v, Wq, bq, Wk, bk, Wv, bv, Wo, bo, trace=False):
    global last_exec_time_ns, last_results
    q = np.asarray(q, dtype=np.float32)
    k = np.asarray(k, dtype=np.float32)
    v = np.asarray(v, dtype=np.float32)
    Wq, Wk, Wv, Wo = (np.asarray(x, dtype=np.float32) for x in (Wq, Wk, Wv, Wo))
    bq, bk, bv, bo = (np.asarray(x, dtype=np.float32) for x in (bq, bk, bv, bo))

    has_bq, has_bk, has_bv, has_bo = (bool(np.any(x)) for x in (bq, bk, bv, bo))

    _install_ntff_shim()
    nc = _build(has_bq, has_bk, has_bv, has_bo)

    in_maps = []
    for c in range(8):
        b, g = divmod(c, 2)
        sl = slice(g * S, (g + 1) * S)
        def tile_em(x):
            # [ET*128, W] -> [128, ET*W]: partition-major tiling for fat DMA lines
            et = x.shape[0] // 128
            return np.ascontiguousarray(
                x.reshape(et, 128, x.shape[1]).transpose(1, 0, 2).reshape(128, -1))

        m = {
            "qT": _bf16c(tile_em(q[b].T)),
            "kT": _bf16c(tile_em(k[b].T)),
            "vT": _bf16c(tile_em(v[b].T)),
            "wq": _bf16c(tile_em(Wq[:, sl])),
            "wk": _bf16c(tile_em(Wk[:, sl])),
            "wv": _bf16c(tile_em(Wv[:, sl])),
            "wo": _bf16c(tile_em(Wo[sl, :])),
        }
        if has_bq:
            m["bq"] = _bf16c(bq[sl].reshape(1, S))
        if has_bk:
            m["bk"] = _bf16c(bk[sl].reshape(1, S))
        if has_bv:
            m["bv"] = _bf16c(bv[sl].reshape(1, S))
        if has_bo:
            m["bo"] = _bf16c((bo if g == 0 else np.zeros_like(bo)).reshape(1, E))
        in_maps.append(m)

    res = run_bass_kernel_spmd(nc, in_maps, core_ids=list(range(8)), trace=trace)
    last_results = res
    last_exec_time_ns = res.exec_time_ns

    out = np.empty((B, N, E), dtype=np.float32)
    for b in range(B):
        out[b] = (res.results[2 * b]["out"] + res.results[2 * b + 1]["out"]).T
    return out



# revision 33
# speedup vs baseline: 1.1550x; 1.0071x over previous
"""Distributed multi-head attention kernel for 8 Trainium2 NeuronCores.

Problem: B=4, N=2048, E=1024, H=16 heads (head_dim 64), QKV + out projections.
Sharding: core c handles batch b=c//2 and head-group g=c%2 (8 heads = D-slice
of 512). QKV projections are column-sharded, the out projection is row-sharded;
the two partial outputs per batch are summed on the host during unshard.

Per-core dataflow (all matmuls bf16 with fp32 PSUM accumulation):
  A) QpT/KpT [512, 2048] and Vp [2048, 512] projections. Host pre-transposes
     q/k/v to [E, N] so the contraction dim lands on SBUF partitions.
  B) S^T[j, i] = Kp_h^T.T @ Qp_h^T per head. K=64, so head pairs are packed
     onto PE row-groups 0-63 / 64-127 (partition-base-derived tile_position).
     exp(scale*x) is fused into the PSUM->SBUF eviction on the scalar engine
     (no max-subtraction: logits are O(1) by construction).
  C) O^T_aug[65, i] accumulates Vp_aug^T @ expS^T over j-tiles, where Vp is
     augmented with a ones column so row 64 of the product is the softmax
     denominator Z.
  D) Normalize: evict O_aug to SBUF (frees the PSUM bank early), DMA the Z row
     to partition 0, reciprocal_approx_fast, gpsimd partition_broadcast, one
     multiply per head; stack head pairs (one SBUF->SBUF DMA partition shift),
     out-project, store out^T partial [1024, 2048].
"""

import sys

import numpy as np


def _ensure_paths():
    try:
        import concourse.bass  # noqa: F401
    except ImportError:
        for p in ("/opt/trn_rl_repo",):
            if p not in sys.path:
                sys.path.insert(0, p)
        import concourse.bass  # noqa: F401


_ensure_paths()

import ml_dtypes  # noqa: E402
import concourse.bass as bass  # noqa: E402
import concourse.bacc as bacc  # noqa: E402
import concourse.mybir as mybir  # noqa: E402
import concourse.tile as tile  # noqa: E402
from concourse.bass_utils import run_bass_kernel_spmd  # noqa: E402

BF16NP = ml_dtypes.bfloat16

B, N, E = 4, 2048, 1024
H, HD = 16, 64
G = 2                 # head-group (tensor-parallel) factor
S = E // G            # 512: per-core slice of the internal dim
HPC = H // G          # 8 heads per core
ET = E // 128         # 8 contraction tiles for the projections
DT = S // 128         # 4 d-tiles per core (= head pairs)
NT = N // 128         # 16 n-tiles
NBLK = N // 512       # 4 n/i blocks
SCALE = 1.0 / float(np.sqrt(HD))
# Schraudolph exp in bf16-bit space: bf16_bits(exp(s*SCALE)) ~= s*EXP_A + EXP_B
# (DVE f32->int16 convert rounds to nearest; B tuned for ~zero mean bias)
EXP_A = 128.0 * 1.4426950408889634 * SCALE
EXP_B = 127.0 * 128.0 - 7.2
# j-tiles (of 16 per block) whose exp runs on DVE via Schraudolph; the rest
# use the scalar engine's exact exp. Balances ACT vs DVE load.
DVE_JT = frozenset((1, 4, 6, 9, 11, 14))

last_exec_time_ns = None
last_results = None


def _install_ntff_shim():
    """Register the axon NTFF profile hook bass_utils wants under trace=True."""
    import types

    if "antenv.axon_hooks" in sys.modules:
        return
    mod = types.ModuleType("antenv.axon_hooks")
    _h = [None]
    mod.set_axon_ntff_profile_hook = lambda h: _h.__setitem__(0, h)
    mod.get_axon_ntff_profile_hook = lambda: _h[0]
    try:
        import antenv

        sys.modules["antenv.axon_hooks"] = mod
        antenv.axon_hooks = mod
        from trn_agent_boot.trn_boot import _ntff_profile_via_ctypes

        hook = _ntff_profile_via_ctypes("/opt/axon/libaxon_pjrt.so")
        if hook is not None:
            mod.set_axon_ntff_profile_hook(hook)
    except Exception:
        pass


def _build(has_bq, has_bk, has_bv, has_bo):
    f32 = mybir.dt.float32
    bf16 = mybir.dt.bfloat16
    PSUM = bass.MemorySpace.PSUM

    nc = bacc.Bacc("TRN2", target_bir_lowering=False, debug=False)

    # host pre-tiles all inputs partition-major: [p, et, ...] so each DMA
    # moves one contiguous multi-KB line per partition
    qT_ext = nc.declare_dram_parameter("qT", [128, ET * N], bf16, isOutput=False)
    kT_ext = nc.declare_dram_parameter("kT", [128, ET * N], bf16, isOutput=False)
    vT_ext = nc.declare_dram_parameter("vT", [128, ET * N], bf16, isOutput=False)
    wq_ext = nc.declare_dram_parameter("wq", [128, ET * S], bf16, isOutput=False)
    wk_ext = nc.declare_dram_parameter("wk", [128, ET * S], bf16, isOutput=False)
    wv_ext = nc.declare_dram_parameter("wv", [128, ET * S], bf16, isOutput=False)
    wo_ext = nc.declare_dram_parameter("wo", [128, DT * E], bf16, isOutput=False)
    bq_ext = nc.declare_dram_parameter("bq", [1, S], bf16, isOutput=False) if has_bq else None
    bk_ext = nc.declare_dram_parameter("bk", [1, S], bf16, isOutput=False) if has_bk else None
    bv_ext = nc.declare_dram_parameter("bv", [1, S], bf16, isOutput=False) if has_bv else None
    bo_ext = nc.declare_dram_parameter("bo", [1, E], bf16, isOutput=False) if has_bo else None
    out_ext = nc.declare_dram_parameter("out", [E, N], f32, isOutput=True)

    with tile.TileContext(nc) as tc:
        with (
            tc.tile_pool(name="const", bufs=1) as cpool,
            tc.tile_pool(name="w", bufs=1) as wpool,
            tc.tile_pool(name="inT", bufs=16) as ipool,
            tc.tile_pool(name="proj", bufs=1) as ppool,
            tc.tile_pool(name="es", bufs=6) as espool,
            tc.tile_pool(name="on", bufs=13) as onpool,
            tc.tile_pool(name="nrm", bufs=2) as nrmpool,
            tc.tile_pool(name="nrm1", bufs=1) as nrm1pool,
            tc.tile_pool(name="dst", bufs=4) as dstpool,
            tc.tile_pool(name="ps_s", bufs=2, space=PSUM) as ps_s,
            tc.tile_pool(name="ps_o", bufs=2, space=PSUM) as ps_o,
            tc.tile_pool(name="ps_m", bufs=2, space=PSUM) as ps_m,
        ):
            # ---- constants -------------------------------------------------
            ones_bf = cpool.tile([1, 512], bf16, tag="ones_bf")
            nc.gpsimd.memset(ones_bf[:], 1.0)

            # ---- PE/ACT warmup during the input-DMA window ----------------
            # ~4us of dummy matmuls keep the HAM activity monitor busy so the
            # first real matmuls run at 2.4GHz instead of the cold 1.2GHz,
            # and a dummy exp preloads the ACT exp table set (~2.7us) off the
            # critical path.
            wm_ps = ps_m.tile([128, 512], f32, tag="d")
            for wi in range(20):
                nc.tensor.matmul(
                    wm_ps[0:1, :], ones_bf[0:1, 0:1], ones_bf[0:1, 0:512],
                    start=True, stop=True, skip_group_check=True,
                )
            wm_out = cpool.tile([1, 512], bf16, tag="wm_out")
            nc.vector.tensor_copy(wm_out[0:1, :], wm_ps[0:1, :])
            wm_es = cpool.tile([1, 512], bf16, tag="wm_es")
            nc.scalar.activation(
                wm_es[0:1, :], ones_bf[0:1, :],
                mybir.ActivationFunctionType.Exp, scale=SCALE,
            )

            # ---- weights + biases -----------------------------------------
            # weight tiles declared here, DMAs emitted in first-use order below
            wq_t = wpool.tile([128, ET, S], bf16, tag="wq")
            wk_t = wpool.tile([128, ET, S], bf16, tag="wk")
            wv_t = wpool.tile([128, ET, S], bf16, tag="wv")
            wo_t = wpool.tile([128, DT, E], bf16, tag="wo")

            def load_weight(w_t, ext, ntiles):
                width = ntiles * (w_t.shape[-1])
                nc.sync.dma_start(out=w_t[:, :, :], in_=ext[:, 0:width])

            bias_tiles = {}

            def load_bias(nm, ext, width):
                if ext is not None:
                    bt = cpool.tile([1, width], bf16, tag=nm)
                    nc.sync.dma_start(out=bt[:], in_=ext[:])
                    bias_tiles[nm] = bt

            # ---- persistent activations -----------------------------------
            qpT = ppool.tile([128, DT, N], bf16, tag="qpT")   # [d, n], d-tiles = head pairs
            kpT = ppool.tile([128, DT, N], bf16, tag="kpT")
            vpa = ppool.tile([128, NT, HPC * 65], bf16, tag="vpa")  # per head: 64 V cols + ones col
            nc.gpsimd.memset(vpa[:], 1.0)  # pre-set so the ones columns survive the V copies

            # ---- phase A: projections -------------------------------------
            def load_inT(ext, quarters=False):
                ts = []
                for t in range(ET):
                    it = ipool.tile([128, N], bf16, tag="inT")
                    ts.append(it)
                nchunk = 4 if quarters else 2
                cw = N // nchunk
                for qr in range(nchunk):
                    for t in range(ET):
                        nc.sync.dma_start(
                            out=ts[t][:, qr * cw:(qr + 1) * cw],
                            in_=ext[:, t * N + qr * cw:t * N + (qr + 1) * cw],
                        )
                return ts

            def proj_group_wide(in_tiles, w_t, b_t, dest, dt, half, slot):
                # one [128, 1024] psum group: dest[:, slot, half*1024 : +1024]
                pt = ps_s.tile([128, 1024], f32, tag="s")
                n0 = half * 1024
                for et in range(ET):
                    for sub in range(2):
                        nc.tensor.matmul(
                            pt[:, sub * 512:(sub + 1) * 512],
                            w_t[:, et, dt * 128:(dt + 1) * 128],
                            in_tiles[et][:, n0 + sub * 512: n0 + (sub + 1) * 512],
                            start=(et == 0),
                            stop=(et == ET - 1 and b_t is None),
                        )
                if b_t is not None:
                    for sub in range(2):
                        nc.tensor.matmul(
                            pt[:, sub * 512:(sub + 1) * 512],
                            b_t[0:1, dt * 128:(dt + 1) * 128],
                            ones_bf[0:1, 0:512],
                            start=False, stop=True,
                        )
                nc.vector.tensor_copy(dest[:, slot, n0:n0 + 1024], pt[:, :])

            def proj_group_jit(in_tiles, w_t, b_t, dest, dt, nb, slot, on_act=False):
                # one [128, 512] psum group on tag "d": dest[:, slot, nb*512 : +512]
                pt = ps_m.tile([128, 512], f32, tag="d")
                n0 = nb * 512
                for et in range(ET):
                    nc.tensor.matmul(
                        pt[:, :],
                        w_t[:, et, dt * 128:(dt + 1) * 128],
                        in_tiles[et][:, n0:n0 + 512],
                        start=(et == 0),
                        stop=(et == ET - 1 and b_t is None),
                    )
                if b_t is not None:
                    nc.tensor.matmul(
                        pt[:, :], b_t[0:1, dt * 128:(dt + 1) * 128],
                        ones_bf[0:1, 0:512], start=False, stop=True,
                    )
                if on_act:
                    nc.scalar.copy(dest[:, slot, n0:n0 + 512], pt[:, :])
                else:
                    nc.vector.tensor_copy(dest[:, slot, n0:n0 + 512], pt[:, :])

            # V first (its input tiles release their slots for qT/kT)
            load_weight(wv_t, wv_ext, ET)
            load_bias("bv", bv_ext, S)
            v_tiles = load_inT(vT_ext, quarters=True)
            bv_t = bias_tiles.get("bv")

            def vp_group(nt):
                pt = ps_m.tile([128, 512], f32, tag="d")
                for et in range(ET):
                    nc.tensor.matmul(
                        pt[:, :],
                        v_tiles[et][:, nt * 128:(nt + 1) * 128],
                        wv_t[:, et, :],
                        start=(et == 0),
                        stop=(et == ET - 1 and bv_t is None),
                    )
                if bv_t is not None:
                    nc.tensor.matmul(
                        pt[:, :], ones_bf[0:1, 0:128], bv_t[0:1, :],
                        start=False, stop=True,
                    )
                # scatter heads into the 65-stride layout (ones col untouched);
                # phase A: the scalar engine is otherwise idle here
                dst = vpa[:, nt, :].rearrange("p (h c) -> p h c", c=65)[:, :, 0:64]
                src_ = pt[:, :].rearrange("p (h c) -> p h c", c=64)
                nc.scalar.copy(dst, src_)

            for nt in range(NT):
                vp_group(nt)

            load_weight(wk_t, wk_ext, ET)
            load_bias("bk", bk_ext, S)
            k_tiles = load_inT(kT_ext)
            load_weight(wq_t, wq_ext, ET)
            load_bias("bq", bq_ext, S)
            q_tiles = load_inT(qT_ext)
            load_weight(wo_t, wo_ext, DT)
            load_bias("bo", bo_ext, E)
            bq_t, bk_t = bias_tiles.get("bq"), bias_tiles.get("bk")
            # projections for head-pair 0 up front; later pairs are JIT filler
            # (narrow tag-"d" groups keep the "s" slots free for block 0's B)
            for nb in range(NBLK):
                proj_group_jit(k_tiles, wk_t, bk_t, kpT, 0, nb, 0, on_act=True)
            for nb in range(NBLK):
                proj_group_jit(q_tiles, wq_t, bq_t, qpT, 0, nb, 0, on_act=True)

            # ---- phases B/C/D ---------------------------------------------
            bo_t = bias_tiles.get("bo")
            on_all = [[None] * NBLK for _ in range(DT)]

            def emit_d_group(ibd, etile, on_act=False):
                    pd = ps_m.tile([128, 512], f32, tag="d")
                    for dt in range(DT):
                        nc.tensor.matmul(
                            pd[:, :],
                            wo_t[:, dt, etile * 128:(etile + 1) * 128],
                            on_all[dt][ibd][:, :],
                            start=(dt == 0),
                            stop=(dt == DT - 1 and bo_t is None),
                        )
                    if bo_t is not None:
                        nc.tensor.matmul(
                            pd[:, :],
                            bo_t[0:1, etile * 128:(etile + 1) * 128],
                            ones_bf[0:1, 0:512],
                            start=False, stop=True,
                        )
                    ds = dstpool.tile([128, 512], f32, tag="dst")
                    if on_act:
                        nc.scalar.copy(ds[:, :], pd[:, :])
                    else:
                        nc.vector.tensor_copy(ds[:, :], pd[:, :])
                    nc.sync.dma_start(
                        out=out_ext[etile * 128:(etile + 1) * 128,
                                    ibd * 512:(ibd + 1) * 512],
                        in_=ds[:, :],
                    )

            def emit_d(ibd, on_act=False):
                # out-projection for n-block ibd; evictions alternate DVE/ACT
                for etile in range(ET):
                    emit_d_group(ibd, etile, on_act=(etile % 2 == 1))
            for hp in range(DT):
                for ib in range(NBLK):
                    i0 = ib * 512
                    o_a = ps_o.tile([65, 512], f32, tag="o")
                    o_b = ps_o.tile([65, 512], f32, tag="o")

                    def c_mms(jtc, rhs):
                        # O accumulation for consumed j-tile jtc (software-
                        # pipelined one jt behind B so the PE FIFO never
                        # stalls at the exp fence with ready B work behind it)
                        nc.tensor.matmul(
                            o_a[:, :],
                            vpa[:, jtc, (2 * hp) * 65:(2 * hp) * 65 + 65],
                            rhs[0],
                            start=(jtc == 0), stop=(jtc == NT - 1),
                        )
                        nc.tensor.matmul(
                            o_b[:, :],
                            vpa[:, jtc, (2 * hp + 1) * 65:(2 * hp + 1) * 65 + 65],
                            rhs[1],
                            start=(jtc == 0), stop=(jtc == NT - 1),
                        )

                    # process j-tiles in pairs: 4 B matmuls, 2 exps, then the
                    # previous pair's 4 C matmuls — halves the full-row-after-
                    # packed-pair LDWEIGHTS/drain tax on the PE
                    pend = []
                    for jp in range(NT // 2):
                        sts = []
                        for dj in range(2):
                            jt = 2 * jp + dj
                            st = ps_s.tile([128, 1024], f32, tag="s")
                            # head A on PE rows 0-63, head B on rows 64-127
                            nc.tensor.matmul(
                                st[:, 0:512],
                                kpT[0:64, hp, jt * 128:(jt + 1) * 128],
                                qpT[0:64, hp, i0:i0 + 512],
                                start=True, stop=True,
                            )
                            nc.tensor.matmul(
                                st[:, 512:1024],
                                kpT[64:128, hp, jt * 128:(jt + 1) * 128],
                                qpT[64:128, hp, i0:i0 + 512],
                                start=True, stop=True,
                            )
                            sts.append((jt, st))
                        newpend = []
                        for jt, st in sts:
                            if jt in DVE_JT:
                                # Schraudolph exp on the vector engine
                                esi = espool.tile([128, 1024], mybir.dt.int16, tag="esI", bufs=4)
                                nc.vector.tensor_scalar(
                                    esi[:, :], st[:, :], EXP_A, EXP_B,
                                    mybir.AluOpType.mult, mybir.AluOpType.add,
                                )
                                rhs = (esi[:, 0:512].bitcast(bf16),
                                       esi[:, 512:1024].bitcast(bf16))
                            else:
                                es = espool.tile([128, 1024], bf16, tag="es")
                                nc.scalar.activation(
                                    es[:], st[:], mybir.ActivationFunctionType.Exp,
                                    scale=SCALE,
                                )
                                rhs = (es[:, 0:512], es[:, 512:1024])
                            newpend.append((jt, rhs))
                        for jtc, rhs in pend:
                            c_mms(jtc, rhs)
                        pend = newpend
                    for jtc, rhs in pend:
                        c_mms(jtc, rhs)
                    # normalize: onorm[0:64] = O_A/Z_A, onorm[64:128] = O_B/Z_B
                    onorm = onpool.tile([128, 512], bf16, tag="onorm")
                    oc_a = nrmpool.tile([65, 512], f32, tag="oc")
                    nc.vector.tensor_copy(oc_a[:, :], o_a[:, :])  # frees psum bank
                    oc_b = nrmpool.tile([65, 512], f32, tag="oc")
                    nc.vector.tensor_copy(oc_b[:, :], o_b[:, :])
                    zr_a = nrm1pool.tile([1, 512], f32, tag="zr")
                    nc.sync.dma_start(out=zr_a[0:1, :], in_=oc_a[64:65, :])
                    zr_b = nrm1pool.tile([1, 512], f32, tag="zr")
                    nc.sync.dma_start(out=zr_b[0:1, :], in_=oc_b[64:65, :])
                    zi_a = nrm1pool.tile([1, 512], f32, tag="zi")
                    nc.vector.reciprocal_approx_fast(zi_a[0:1, :], zr_a[0:1, :])
                    zi_b = nrm1pool.tile([1, 512], f32, tag="zi")
                    nc.vector.reciprocal_approx_fast(zi_b[0:1, :], zr_b[0:1, :])
                    zb_a = nrmpool.tile([64, 512], f32, tag="zb")
                    nc.gpsimd.partition_broadcast(zb_a[:, :], zi_a[0:1, :])
                    zb_b = nrmpool.tile([64, 512], f32, tag="zb")
                    nc.gpsimd.partition_broadcast(zb_b[:, :], zi_b[0:1, :])
                    nc.vector.tensor_mul(onorm[0:64, :], oc_a[0:64, :], zb_a[:, :])
                    tmp_b = nrm1pool.tile([64, 512], bf16, tag="tmpB")
                    nc.vector.tensor_mul(tmp_b[:, :], oc_b[0:64, :], zb_b[:, :])
                    # partition shift 0-63 -> 64-127 (DMA crosses partitions)
                    nc.sync.dma_start(out=onorm[64:128, :], in_=tmp_b[:, :])
                    on_all[hp][ib] = onorm

                    if hp == DT - 1 and ib > 0:
                        # phase D one block behind (its matmuls fence on the
                        # normalize chain; keep ready B/C work ahead of them);
                        # the last of these also runs post-exp -> ACT evictions
                        emit_d(ib - 1, on_act=(ib == NBLK - 1))
                    if hp < DT - 1:
                        # JIT projections for the next head pair: PE filler
                        # that keeps the array busy while ACT drains exps.
                        proj_group_jit(k_tiles, wk_t, bk_t, kpT, hp + 1, ib,
                                       hp + 1, on_act=(ib % 2 == 0))
                        proj_group_jit(q_tiles, wq_t, bq_t, qpT, hp + 1, ib,
                                       hp + 1, on_act=(ib % 2 == 1))

                if hp == DT - 1:
                    # the final block runs after the last exp: the scalar
                    # engine is idle, so route its evictions there
                    emit_d(NBLK - 1, on_act=True)

    nc.compile()
    return nc


def _bf16c(a):
    return np.ascontiguousarray(a, dtype=np.float32).astype(BF16NP)


def kernel(q, k, v, Wq, bq, Wk, bk, Wv, bv, Wo, bo, trace=False):
    global last_exec_time_ns, last_results
    q = np.asarray(q, dtype=np.float32)
    k = np.asarray(k, dtype=np.float32)
    v = np.asarray(v, dtype=np.float32)
    Wq, Wk, Wv, Wo = (np.asarray(x, dtype=np.float32) for x in (Wq, Wk, Wv, Wo))
    bq, bk, bv, bo = (np.asarray(x, dtype=np.float32) for x in (bq, bk, bv, bo))

    has_bq, has_bk, has_bv, has_bo = (bool(np.any(x)) for x in (bq, bk, bv, bo))

    _install_ntff_shim()
    nc = _build(has_bq, has_bk, has_bv, has_bo)

    in_maps = []
    for c in range(8):
        b, g = divmod(c, 2)
        sl = slice(g * S, (g + 1) * S)
        def tile_em(x):
            # [ET*128, W] -> [128, ET*W]: partition-major tiling for fat DMA lines
            et = x.shape[0] // 128
            return np.ascontiguousarray(
                x.reshape(et, 128, x.shape[1]).transpose(1, 0, 2).reshape(128, -1))

        m = {
            "qT": _bf16c(tile_em(q[b].T)),
            "kT": _bf16c(tile_em(k[b].T)),
            "vT": _bf16c(tile_em(v[b].T)),
            "wq": _bf16c(tile_em(Wq[:, sl])),
            "wk": _bf16c(tile_em(Wk[:, sl])),
            "wv": _bf16c(tile_em(Wv[:, sl])),
            "wo": _bf16c(tile_em(Wo[sl, :])),
        }
        if has_bq:
            m["bq"] = _bf16c(bq[sl].reshape(1, S))
        if has_bk:
            m["bk"] = _bf16c(bk[sl].reshape(1, S))
        if has_bv:
            m["bv"] = _bf16c(bv[sl].reshape(1, S))
        if has_bo:
            m["bo"] = _bf16c((bo if g == 0 else np.zeros_like(bo)).reshape(1, E))
        in_maps.append(m)

    res = run_bass_kernel_spmd(nc, in_maps, core_ids=list(range(8)), trace=trace)
    last_results = res
    last_exec_time_ns = res.exec_time_ns

    out = np.empty((B, N, E), dtype=np.float32)
    for b in range(B):
        out[b] = (res.results[2 * b]["out"] + res.results[2 * b + 1]["out"]).T
    return out

